# revision 1
# baseline (speedup 1.0000x reference)
"""BlockSparseAttention TRN2 kernel — 8-core SPMD (batch x head-half sharding).

v2: single interleaved schedule (no phases). Projections split into q/k/v
passes of 4 PSUM banks each; attention units (one per (r, hp) s-tile x
head-pair, r order [1..7, 0]) and o-projection 128-row blocks are woven
between passes so the PE never idles long enough for the HAM clock gate to
drop to 4/8. x is SBUF-resident (DMA'd once, weight-first priority order).

Per-unit attention: scores for each head-half chunked over 128-wide t
blocks into one PSUM bank; exp first (scalar), then 0/1-mask multiply
(DVE, bf16); AV accumulates into an aug bank whose rows 0:64 hold V-
weighted sums, row 64 the softmax denominators (ones column in v), rows
96:100 the global-column strip scores. Normalization divides via
reciprocal + partition-broadcast multiply straight out of PSUM.

Core c: batch b=c//2, head-group g=c%2 (heads 8g..8g+7, channels
512g..512g+511). Each core emits a bf16 partial o-projection; the host
sums the two partials per batch and adds bo.
"""
import numpy as np
import ml_dtypes

import concourse.bass as bass
import concourse.bacc as bacc
import concourse.tile as tile
import concourse.mybir as mybir
from concourse.bass_utils import run_bass_kernel_spmd

F32 = mybir.dt.float32
BF16 = mybir.dt.bfloat16
AF = mybir.ActivationFunctionType
ALU = mybir.AluOpType

S = 2048
D = 1024
NCORES = 8
SCALE = 0.125


def chunk_plan():
    """Per s-tile r (256 rows): list of (t0, lo, w): 128-wide t-chunk at t0,
    contributing to local s columns [lo, lo+w)."""
    plans = []
    for r in range(8):
        if r == 0:
            t0s = [0, 128, 256]
        elif r == 7:
            t0s = [1664, 1792, 1920]
        else:
            t0s = [256 * r - 128, 256 * r, 256 * r + 128, 256 * r + 256]
        cur = []
        for j, t0 in enumerate(t0s):
            if r == 0 and j == 0:
                lo, hi = 0, 256  # global cols t<4 make every s valid
            else:
                lo = max(0, t0 - 32 - 256 * r)
                hi = min(256, t0 + 160 - 256 * r)
            cur.append((t0, lo, hi - lo))
        plans.append(cur)
    return plans


PLANS = chunk_plan()
MASK_OFF = []  # flat offsets into packed masks tensor, in (r, j) order
_off = 0
for _r in range(8):
    _row = []
    for (_t0, _lo, _w) in PLANS[_r]:
        _row.append(_off)
        _off += _w
    MASK_OFF.append(_row)
MASK_W = _off  # total packed width

R_ORDER = [1, 2, 3, 4, 5, 6, 7, 0]
UNITS = [(r, hp) for r in R_ORDER for hp in range(4)]  # 32 units
UNIT_IDX_OF_R = {r: [i for i, (ur, _) in enumerate(UNITS) if ur == r]
                 for r in range(8)}
# o-proj st -> last unit index it depends on (r pair {2st, 2st+1})
OP_DEP = {st: max(UNIT_IDX_OF_R[2 * st] + UNIT_IDX_OF_R[2 * st + 1])
          for st in range(4)}
# op emission order: st by readiness (st0 last since it needs r0)
OP_LIST = [(st, et) for st in (1, 2, 3, 0) for et in range(8)]


def build_nc():
    nc = bacc.Bacc()
    xT = nc.dram_tensor("xT", [128, 8, S], BF16, kind="ExternalInput")
    wq = nc.dram_tensor("wq", [128, 8, 512], BF16, kind="ExternalInput")
    wk = nc.dram_tensor("wk", [128, 8, 512], BF16, kind="ExternalInput")
    wv = nc.dram_tensor("wv", [128, 8, 512], BF16, kind="ExternalInput")
    wo = nc.dram_tensor("wo", [128, 4, 1024], BF16, kind="ExternalInput")
    bq_c = nc.dram_tensor("bq_c", [128, 4], F32, kind="ExternalInput")
    bk_c = nc.dram_tensor("bk_c", [128, 4], F32, kind="ExternalInput")
    bv_b = nc.dram_tensor("bv_b", [128, 512], F32, kind="ExternalInput")
    masks = nc.dram_tensor("masks", [128, 2 * MASK_W], BF16,
                           kind="ExternalInput")
    out = nc.dram_tensor("out", [128, 8, S], BF16, kind="ExternalOutput")
    import os as _os
    DBG = _os.environ.get("KDBG", "")
    dbg = None
    if DBG:
        dbg = nc.dram_tensor("dbg", [128, 4, S], BF16, kind="ExternalOutput")

    with tile.TileContext(nc) as tc:
        with (
            tc.tile_pool(name="pers", bufs=1) as pers,
            tc.tile_pool(name="small", bufs=1) as small,
            tc.tile_pool(name="epool", bufs=3) as epool,
            tc.tile_pool(name="rpool", bufs=5) as rpool,
            tc.tile_pool(name="opool", bufs=3) as opool,
        ):
            xT_sb = pers.tile([128, 8, S], BF16)
            wq_sb = pers.tile([128, 8, 512], BF16)
            wk_sb = pers.tile([128, 8, 512], BF16)
            wv_sb = pers.tile([128, 8, 512], BF16)
            wo_sb = pers.tile([128, 4, 1024], BF16)
            masks_sb = pers.tile([128, 2 * MASK_W], BF16)
            q_sb = pers.tile([128, 4, S], BF16)
            k_sb = pers.tile([128, 4, S], BF16)
            v_sb = pers.tile([128, 16, 520], BF16)
            att_sb = pers.tile([128, 4, S], BF16)
            bq_sb = small.tile([128, 4], F32)
            bk_sb = small.tile([128, 4], F32)
            bv_sb = small.tile([128, 512], F32)

            # ---- DMA emission, priority order. Few large dma_starts: each
            # costs ~660ns of SP issue time.
            nc.sync.dma_start(out=bk_sb, in_=bk_c.ap())
            nc.sync.dma_start(out=wk_sb[:, 0:2, :], in_=wk.ap()[:, 0:2, :])
            nc.sync.dma_start(out=xT_sb[:, 0:2, 0:512],
                              in_=xT.ap()[:, 0:2, 0:512])
            nc.sync.dma_start(out=wk_sb[:, 2:4, :], in_=wk.ap()[:, 2:4, :])
            nc.sync.dma_start(out=xT_sb[:, 2:4, 0:512],
                              in_=xT.ap()[:, 2:4, 0:512])
            nc.sync.dma_start(out=wk_sb[:, 4:8, :], in_=wk.ap()[:, 4:8, :])
            nc.sync.dma_start(out=xT_sb[:, 4:8, 0:512],
                              in_=xT.ap()[:, 4:8, 0:512])
            nc.sync.dma_start(out=bq_sb, in_=bq_c.ap())
            nc.sync.dma_start(out=wq_sb, in_=wq.ap())
            nc.sync.dma_start(out=bv_sb, in_=bv_b.ap())
            nc.sync.dma_start(out=wv_sb, in_=wv.ap())
            nc.sync.dma_start(out=xT_sb[:, :, 512:1024],
                              in_=xT.ap()[:, :, 512:1024])
            nc.sync.dma_start(out=masks_sb, in_=masks.ap())
            nc.sync.dma_start(out=xT_sb[:, :, 1024:1536],
                              in_=xT.ap()[:, :, 1024:1536])
            nc.sync.dma_start(out=wo_sb, in_=wo.ap())
            nc.sync.dma_start(out=xT_sb[:, :, 1536:2048],
                              in_=xT.ap()[:, :, 1536:2048])

            unit_state = {}
            exg_st = {}

            def qk_pass(pool, st, w_sb, b_sb, dst, nm):
                ssl = slice(st * 512, (st + 1) * 512)
                acc = [pool.tile([128, 512], F32, tag="proj",
                                 name=f"p{nm}{st}_{i}") for i in range(4)]
                for dc in range(8):
                    for cb in range(4):
                        csl = slice(cb * 128, (cb + 1) * 128)
                        nc.tensor.matmul(
                            acc[cb], w_sb[:, dc, csl], xT_sb[:, dc, ssl],
                            start=(dc == 0), stop=(dc == 7))
                for cb in range(4):
                    nc.scalar.activation(
                        out=dst[:, cb, ssl], in_=acc[cb], func=AF.Identity,
                        bias=b_sb[:, cb:cb + 1], scale=1.0)

            def v_pass(pool, st):
                ssl0 = st * 512
                pv = [pool.tile([128, 512], F32, tag="proj",
                                name=f"pv{st}_{i}") for i in range(4)]
                for dc in range(8):
                    for s4 in range(4):
                        nc.tensor.matmul(
                            pv[s4],
                            xT_sb[:, dc, ssl0 + s4 * 128:ssl0 + (s4 + 1) * 128],
                            wv_sb[:, dc, :],
                            start=(dc == 0), stop=(dc == 7))
                for s4 in range(4):
                    sc = st * 4 + s4
                    vview = v_sb[:, sc, :].rearrange("p (h w) -> p h w", h=8)
                    nc.vector.tensor_add(
                        out=vview[:, :, 0:64],
                        in0=pv[s4].rearrange("p (h w) -> p h w", h=8),
                        in1=bv_sb.rearrange("p (h w) -> p h w", h=8))
                    nc.gpsimd.memset(vview[:, :, 64:65], 1.0)

            def gsc_batch(pool, st):
                # global-row scores for k-pass st; packs 8 (hp,hs) x 4 kk:
                # col = (2*hp + hs//64)*16 + (kk%4)*4
                g = pool.tile([128, 128], F32, tag="sc", name=f"gsc{st}")
                first = True
                for hp in range(4):
                    for hs in (0, 64):
                        base = (2 * hp + hs // 64) * 16
                        for kk in range(4 * st, 4 * st + 4):
                            c = base + (kk % 4) * 4
                            nc.tensor.matmul(
                                g[:, c:c + 4],
                                k_sb[hs:hs + 64, hp, 128 * kk:128 * kk + 128],
                                q_sb[hs:hs + 64, hp, 0:4],
                                start=first,
                                stop=(hp == 3 and hs == 64
                                      and kk == 4 * st + 3),
                                skip_group_check=True)
                            first = False
                ex = epool.tile([128, 128], BF16, tag="exg", bufs=4,
                                name=f"exg{st}")
                nc.scalar.activation(ex, g, AF.Exp, scale=SCALE)
                exg_st[st] = ex

            def S_unit(i, scp, augp):
                r, hp = UNITS[i]
                rsl = slice(r * 256, (r + 1) * 256)
                offs = []
                acc = 0
                for (_t0, _lo, _w) in PLANS[r]:
                    offs.append(acc)
                    acc += _w
                wr = acc
                moff = MASK_OFF[r][0]
                aug = augp.tile([128, 512], F32, tag="aug", name=f"au{i}")
                st_ = {"aug": aug, "offs": offs, "wr": wr, "exs": {}}
                if r > 0:
                    strips = {}
                    for hs in (0, 64):
                        strips[hs] = scp.tile([4, 256], F32, tag="sc",
                                              name=f"st{i}{hs}")
                        nc.tensor.matmul(
                            strips[hs], k_sb[hs:hs + 64, hp, 0:4],
                            q_sb[hs:hs + 64, hp, rsl],
                            start=True, stop=True)
                    for hs in (0, 64):
                        exs = epool.tile([4, 256], BF16, tag="exs",
                                         bufs=4, name=f"xs{i}{hs}")
                        nc.scalar.activation(exs, strips[hs], AF.Exp,
                                             scale=SCALE)
                        st_["exs"][hs] = exs
                else:
                    st_["exg"] = {}
                    for hs in (0, 64):
                        gsc = scp.tile([128, 64], F32, tag="sc",
                                       name=f"gs{i}{hs}")
                        for kk in range(16):
                            nc.tensor.matmul(
                                gsc[:, 4 * kk:4 * kk + 4],
                                k_sb[hs:hs + 64, hp, 128 * kk:128 * kk + 128],
                                q_sb[hs:hs + 64, hp, 0:4],
                                start=(kk == 0), stop=(kk == 15),
                                skip_group_check=True)
                        exg = epool.tile([128, 64], BF16, tag="exg",
                                         bufs=4, name=f"xg{i}{hs}")
                        nc.scalar.activation(exg, gsc, AF.Exp, scale=SCALE)
                        st_["exg"][hs] = exg
                # band scores: the two head-halves interleaved chunk-by-chunk
                # (disjoint PE row groups run concurrently).
                scts = {}
                for hs in (0, 64):
                    scts[hs] = scp.tile([128, 512], F32, tag="sc",
                                        name=f"sc{i}{hs}")
                for j, (t0, lo, w) in enumerate(PLANS[r]):
                    ssl2 = slice(r * 256 + lo, r * 256 + lo + w)
                    for hs in (0, 64):
                        nc.tensor.matmul(
                            scts[hs][:, offs[j]:offs[j] + w],
                            k_sb[hs:hs + 64, hp, t0:t0 + 128],
                            q_sb[hs:hs + 64, hp, ssl2],
                            start=(j == 0), stop=(j == len(PLANS[r]) - 1),
                            skip_group_check=True)
                ex2 = epool.tile([128, 1024], BF16, tag="ex",
                                 bufs=4, name=f"ex{i}")
                for hs in (0, 64):
                    nc.scalar.activation(
                        ex2[:, (hs // 64) * wr:(hs // 64) * wr + wr],
                        scts[hs][:, 0:wr], AF.Exp, scale=SCALE)
                exm2 = epool.tile([128, 1024], BF16, tag="exm",
                                  bufs=6, name=f"xm{i}")
                nc.vector.tensor_mul(
                    out=exm2[:, 0:2 * wr], in0=ex2[:, 0:2 * wr],
                    in1=masks_sb[:, 2 * moff:2 * moff + 2 * wr])
                st_["exm"] = exm2
                unit_state[i] = st_

            def A_unit(i):
                r, hp = UNITS[i]
                rsl = slice(r * 256, (r + 1) * 256)
                st_ = unit_state.pop(i)
                aug, offs, wr = st_["aug"], st_["offs"], st_["wr"]
                started = False
                for hs in (0, 64):
                    half = (hs // 64) * 256
                    h65 = (hp * 2 + hs // 64) * 65
                    exm = st_["exm"][:, (hs // 64) * wr:(hs // 64) * wr + wr]
                    if r > 0:
                        nc.tensor.matmul(
                            aug[0:65, half:half + 256],
                            v_sb[0:4, 0, h65:h65 + 65], st_["exs"][hs],
                            start=not started, stop=False,
                            skip_group_check=True)
                        started = True
                    for j, (t0, lo, w) in enumerate(PLANS[r]):
                        last = (r > 0 and hs == 64 and j == len(PLANS[r]) - 1)
                        nc.tensor.matmul(
                            aug[0:65, half + lo:half + lo + w],
                            v_sb[:, t0 // 128, h65:h65 + 65],
                            exm[:, offs[j]:offs[j] + w],
                            start=not started, stop=last,
                            skip_group_check=True)
                        started = True
                    if r == 0:
                        exg = st_["exg"][hs]
                        for kk in range(16):
                            nc.tensor.matmul(
                                aug[0:65, half:half + 4],
                                v_sb[:, kk, h65:h65 + 65],
                                exg[:, 4 * kk:4 * kk + 4],
                                start=False,
                                stop=(hs == 64 and kk == 15),
                                skip_group_check=True)
                # normalization: att = aug[0:64] * (1 / aug[64])
                sums = rpool.tile([1, 512], F32, tag="sums", name=f"su{i}")
                nc.vector.tensor_copy(out=sums, in_=aug[64:65, :])
                rec = rpool.tile([1, 512], F32, tag="rec", name=f"re{i}")
                nc.vector.reciprocal_approx_fast(out=rec, in_=sums)
                bc = rpool.tile([128, 512], F32, tag="bc", name=f"bc{i}")
                nc.gpsimd.partition_broadcast(bc, rec)
                for hs in (0, 64):
                    half = (hs // 64) * 256
                    if DBG == "raw":
                        nc.vector.tensor_copy(
                            out=att_sb[hs:hs + 64, hp, rsl],
                            in_=aug[0:64, half:half + 256])
                    elif DBG == "den":
                        nc.vector.tensor_copy(
                            out=att_sb[hs:hs + 64, hp, rsl],
                            in_=bc[hs:hs + 64, half:half + 256])
                    else:
                        nc.vector.tensor_mul(
                            out=att_sb[hs:hs + 64, hp, rsl],
                            in0=aug[0:64, half:half + 256],
                            in1=bc[hs:hs + 64, half:half + 256])

            def op_et(pool, st, et, tag="po"):
                ssl = slice(st * 512, (st + 1) * 512)
                esl = slice(et * 128, (et + 1) * 128)
                po = pool.tile([128, 512], F32, tag=tag, name=f"po{st}_{et}")
                for cb in range(4):
                    nc.tensor.matmul(
                        po, wo_sb[:, cb, esl], att_sb[:, cb, ssl],
                        start=(cb == 0), stop=(cb == 3))
                otq = opool.tile([128, 512], BF16, tag="otq",
                                 name=f"otq{st}_{et}")
                nc.any.tensor_copy(out=otq, in_=po)
                nc.sync.dma_start(out=out.ap()[:, et, ssl], in_=otq)

            # ---------------- proj region ----------------
            with (
                tc.tile_pool(name="pproj", bufs=4, space="PSUM") as pproj,
                tc.tile_pool(name="psc", bufs=2, space="PSUM") as psc,
                tc.tile_pool(name="paug", bufs=2, space="PSUM") as paug,
            ):
                def S_(i):
                    S_unit(i, psc, paug)
                qk_pass(pproj, 0, wk_sb, bk_sb, k_sb, "k")
                qk_pass(pproj, 0, wq_sb, bq_sb, q_sb, "q")
                v_pass(pproj, 0)
                qk_pass(pproj, 1, wk_sb, bk_sb, k_sb, "k")
                S_(0)
                qk_pass(pproj, 1, wq_sb, bq_sb, q_sb, "q")
                S_(1)
                v_pass(pproj, 1)
                A_unit(0); S_(2); A_unit(1); S_(3)
                qk_pass(pproj, 2, wk_sb, bk_sb, k_sb, "k")
                A_unit(2); S_(4); A_unit(3); S_(5)
                v_pass(pproj, 2)
                A_unit(4); S_(6); A_unit(5); S_(7)
                qk_pass(pproj, 2, wq_sb, bq_sb, q_sb, "q")
                A_unit(6); S_(8); A_unit(7); S_(9)
                qk_pass(pproj, 3, wk_sb, bk_sb, k_sb, "k")
                A_unit(8); S_(10); A_unit(9); S_(11)
                v_pass(pproj, 3)
                A_unit(10); S_(12); A_unit(11); S_(13); A_unit(12); S_(14)
                qk_pass(pproj, 3, wq_sb, bq_sb, q_sb, "q")
                A_unit(13); S_(15); A_unit(14); S_(16); A_unit(15); S_(17)
                A_unit(16); A_unit(17)

            # ---------------- tail ----------------
            with (
                tc.tile_pool(name="psc2", bufs=3, space="PSUM") as psc2,
                tc.tile_pool(name="paug2", bufs=3, space="PSUM") as paug2,
                tc.tile_pool(name="ptail", bufs=2, space="PSUM") as ptail,
            ):
                nxt = 0

                def avail(dep):
                    n = 0
                    k = nxt
                    while (k < len(OP_LIST)
                           and OP_DEP[OP_LIST[k][0]] <= dep):
                        n += 1
                        k += 1
                    return n

                for i in range(18, 32):
                    if i >= 20:
                        A_unit(i - 2)
                    a = avail(i - 2 if i >= 20 else 17)
                    remaining = 32 - i
                    take = min(2, a, max(1, -(-a // max(1, remaining))))
                    for _ in range(take):
                        op_et(ptail, *OP_LIST[nxt])
                        nxt += 1
                    S_unit(i, psc2, paug2)
                A_unit(30)
                for _ in range(min(2, avail(30))):
                    op_et(ptail, *OP_LIST[nxt])
                    nxt += 1
                A_unit(31)
                while nxt < len(OP_LIST):
                    op_et(ptail, *OP_LIST[nxt])
                    nxt += 1

            if DBG in ("att", "raw", "den"):
                nc.sync.dma_start(out=dbg.ap(), in_=att_sb)
            elif DBG == "q":
                nc.sync.dma_start(out=dbg.ap(), in_=q_sb)
            elif DBG == "k":
                nc.sync.dma_start(out=dbg.ap(), in_=k_sb)
            elif DBG == "v":
                nc.sync.dma_start(out=dbg.ap()[:, :, 0:520], in_=v_sb[:, 0:4, :])
                nc.sync.dma_start(out=dbg.ap()[:, :, 520:1040], in_=v_sb[:, 4:8, :])
                nc.sync.dma_start(out=dbg.ap()[:, :, 1040:1560], in_=v_sb[:, 8:12, :])
                nc.sync.dma_start(out=dbg.ap()[:, :, 1560:2048], in_=v_sb[:, 12:16, 0:488])

    nc.compile()
    return nc


def _host_masks():
    """0/1 multiplicative masks (applied to exp'd scores), packed per chunk.
    Each r block is duplicated so one DVE mul covers both head-halves."""
    p = np.arange(128)[:, None]
    tiles = np.empty((128, MASK_W), np.float32)
    for r in range(8):
        for j, (t0, lo, w) in enumerate(PLANS[r]):
            sl = np.arange(lo, lo + w)[None, :]
            s = 256 * r + sl
            t = t0 + p
            valid = (s >= 4) & ((np.abs(t - s) <= 32) | (t < 4))
            mo = MASK_OFF[r][j]
            tiles[:, mo:mo + w] = np.where(valid, 1.0, 0.0)
    out = np.empty((128, 2 * MASK_W), np.float32)
    for r in range(8):
        mo = MASK_OFF[r][0]
        wr = sum(w for (_t, _l, w) in PLANS[r])
        out[:, 2 * mo:2 * mo + wr] = tiles[:, mo:mo + wr]
        out[:, 2 * mo + wr:2 * mo + 2 * wr] = tiles[:, mo:mo + wr]
    return out.astype(ml_dtypes.bfloat16)


_NC = None
_LAST_IN_MAPS = None
_LAST_RES = None


def kernel(x, Wq, bq, Wk, bk, Wv, bv, Wo, bo):
    global _NC
    if _NC is None:
        _NC = build_nc()
    nc = _NC
    x = np.asarray(x, np.float32)
    B = x.shape[0]
    bf = ml_dtypes.bfloat16

    def chunked_T(a):  # [R, C] -> [128, C//128, R]; [p, c, r] = a[r, 128c+p]
        at = np.ascontiguousarray(a.T)
        return at.reshape(at.shape[0] // 128, 128, at.shape[1]).transpose(1, 0, 2)

    masks_h = _host_masks()
    in_maps = []
    for core in range(NCORES):
        b, g = core // 2, core % 2
        gs = slice(512 * g, 512 * (g + 1))
        in_maps.append({
            "xT": np.ascontiguousarray(chunked_T(x[b])).astype(bf),
            "wq": np.ascontiguousarray(chunked_T(np.asarray(Wq)[gs, :])).astype(bf),
            "wk": np.ascontiguousarray(chunked_T(np.asarray(Wk)[gs, :])).astype(bf),
            "wv": np.ascontiguousarray(chunked_T(np.asarray(Wv)[gs, :])).astype(bf),
            "wo": np.ascontiguousarray(chunked_T(np.asarray(Wo)[:, gs])).astype(bf),
            "bq_c": np.asarray(bq)[gs].reshape(4, 128).T.copy().astype(np.float32),
            "bk_c": np.asarray(bk)[gs].reshape(4, 128).T.copy().astype(np.float32),
            "bv_b": np.broadcast_to(
                np.asarray(bv)[gs], (128, 512)).copy().astype(np.float32),
            "masks": masks_h,
        })

    global _LAST_IN_MAPS, _LAST_RES
    _LAST_IN_MAPS = in_maps
    res = run_bass_kernel_spmd(nc, in_maps, list(range(NCORES)))
    _LAST_RES = res
    out = np.empty((B, S, D), np.float32)
    for b in range(B):
        acc = res.results[2 * b]["out"].astype(np.float32) + \
            res.results[2 * b + 1]["out"].astype(np.float32)
        full_T = acc.transpose(1, 0, 2).reshape(D, S)
        out[b] = full_T.T + np.asarray(bo)[None, :]
    return out



# revision 9
# speedup vs baseline: 1.0393x; 1.0393x over previous
"""BlockSparseAttention TRN2 kernel — 8-core SPMD (batch x head-half sharding).

v3: t-chunk-centric attention. 16 t-chunks of 128 (score s-windows of
<=192), processed as 8 chunk-pairs per head-pair: both band chunks + the
4 global-row score columns (s<4) share one PSUM bank per head-half, so a
single exp activation and a single mask-multiply cover a whole pair.
Global-col strips (t<4) are computed k-global-stationary at N=512 with 4
strips packed per bank at partition offsets {0,32,64,96} (col-tiled) and
exp'd in one activation. AV accumulates per (head-pair, half, 512-s-block)
into a [65,512] bank (row 64 = softmax denominators via the ones column in
v); global-row AV accumulates N=4 matmuls into one persistent bank.
o-projection for s-block 0 covers cols 4:512 so it never waits on the
global rows; cols 0:4 are a tiny tail pass. Projection passes and
o-projection units act as PE filler woven between attention units to keep
the HAM clock gate warm. DMA is split across the SP and ACT HWDGE queues.

Core c: batch b=c//2, head-group g=c%2 (heads 8g..8g+7, channels
512g..512g+511). Each core emits a bf16 partial o-projection; the host
sums the two partials per batch and adds bo.
"""
from collections import deque

import numpy as np
import ml_dtypes

import concourse.bass as bass
import concourse.bacc as bacc
import concourse.tile as tile
import concourse.mybir as mybir
from concourse.bass_utils import run_bass_kernel_spmd

F32 = mybir.dt.float32
BF16 = mybir.dt.bfloat16
AF = mybir.ActivationFunctionType

S = 2048
D = 1024
NCORES = 8
SCALE = 0.125


def chunk_sw(kk):
    if kk == 0:
        return 0, 160
    if kk == 15:
        return 1888, 160
    return 128 * kk - 32, 192


def pair_widths(P):
    wA = chunk_sw(2 * P)[1]
    wB = chunk_sw(2 * P + 1)[1]
    return wA, wB, wA + wB + 8


def msk_off(P):
    if P == 0:
        return 0
    if P == 7:
        return 1504
    return 720


MSK_W = 2224


def av_chunks(st):
    """Band-AV contributions for s-block st: (P, par, lo, w, acol)."""
    out = []
    if st > 0:
        out.append((2 * st - 1, 1, 160, 32, 0))
        out.append((2 * st, 0, 32, 160, 0))
    else:
        out.append((0, 0, 0, 160, 0))
    out.append((2 * st, 1, 0, 192, 96))
    out.append((2 * st + 1, 0, 0, 192, 224))
    out.append((2 * st + 1, 1, 0, 160, 352))
    if st < 3:
        out.append((2 * st + 2, 0, 0, 32, 480))
    return out


def build_nc():
    nc = bacc.Bacc()
    xT = nc.dram_tensor("xT", [128, 8, S], BF16, kind="ExternalInput")
    wq = nc.dram_tensor("wq", [128, 8, 512], BF16, kind="ExternalInput")
    wk = nc.dram_tensor("wk", [128, 8, 512], BF16, kind="ExternalInput")
    wv = nc.dram_tensor("wv", [128, 8, 512], BF16, kind="ExternalInput")
    wo = nc.dram_tensor("wo", [128, 4, 1024], BF16, kind="ExternalInput")
    bq_c = nc.dram_tensor("bq_c", [128, 4], F32, kind="ExternalInput")
    bk_c = nc.dram_tensor("bk_c", [128, 4], F32, kind="ExternalInput")
    bv_b = nc.dram_tensor("bv_b", [128, 512], F32, kind="ExternalInput")
    masks = nc.dram_tensor("masks", [128, MSK_W], BF16, kind="ExternalInput")
    out = nc.dram_tensor("out", [128, 8, S], BF16, kind="ExternalOutput")
    import os as _os
    DBG = _os.environ.get("KDBG", "")
    dbg = None
    if DBG:
        dbg = nc.dram_tensor("dbg", [128, 4, S], BF16, kind="ExternalOutput")

    with tile.TileContext(nc) as tc:
        with (
            tc.tile_pool(name="pers", bufs=1) as pers,
            tc.tile_pool(name="small", bufs=1) as small,
            tc.tile_pool(name="epool", bufs=3) as epool,
            tc.tile_pool(name="rpool", bufs=3) as rpool,
            tc.tile_pool(name="opool", bufs=3) as opool,
            tc.tile_pool(name="pA", bufs=2, space="PSUM") as pA,
            tc.tile_pool(name="pSC", bufs=2, space="PSUM") as pSC,
            tc.tile_pool(name="pAU", bufs=3, space="PSUM") as pAU,
            tc.tile_pool(name="pG", bufs=1, space="PSUM") as pG,
        ):
            xT_sb = pers.tile([128, 8, S], BF16)
            wq_sb = pers.tile([128, 8, 512], BF16)
            wk_sb = pers.tile([128, 8, 512], BF16)
            wv_sb = pers.tile([128, 8, 512], BF16)
            wo_sb = pers.tile([128, 4, 1024], BF16)
            masks_sb = pers.tile([128, MSK_W], BF16)
            q_sb = pers.tile([128, 4, S], BF16)
            k_sb = pers.tile([128, 4, S], BF16)
            v_sb = pers.tile([128, 16, 520], BF16)
            att_sb = pers.tile([128, 4, S], BF16)
            vg_sb = pers.tile([100, 520], BF16)
            bq_sb = small.tile([128, 4], F32)
            bk_sb = small.tile([128, 4], F32)
            bv_sb = small.tile([128, 512], F32)
            zz_sb = small.tile([1, 640], BF16)

            # ---- input DMA, split across the two HWDGE queues ----
            nc.sync.dma_start(out=wk_sb[:, 0:2, :], in_=wk.ap()[:, 0:2, :])
            nc.scalar.dma_start(out=xT_sb[:, 0:2, 0:512],
                                in_=xT.ap()[:, 0:2, 0:512])
            nc.sync.dma_start(out=wk_sb[:, 2:5, :], in_=wk.ap()[:, 2:5, :])
            nc.scalar.dma_start(out=xT_sb[:, 2:5, 0:512],
                                in_=xT.ap()[:, 2:5, 0:512])
            nc.sync.dma_start(out=wk_sb[:, 5:8, :], in_=wk.ap()[:, 5:8, :])
            nc.scalar.dma_start(out=xT_sb[:, 5:8, 0:512],
                                in_=xT.ap()[:, 5:8, 0:512])
            nc.sync.dma_start(out=bk_sb, in_=bk_c.ap())
            nc.sync.dma_start(out=bq_sb, in_=bq_c.ap())
            nc.sync.dma_start(out=wq_sb, in_=wq.ap())
            nc.scalar.dma_start(out=masks_sb, in_=masks.ap())
            nc.sync.dma_start(out=bv_sb, in_=bv_b.ap())
            nc.sync.dma_start(out=wv_sb, in_=wv.ap())
            nc.scalar.dma_start(out=xT_sb[:, :, 512:1024],
                                in_=xT.ap()[:, :, 512:1024])
            nc.sync.dma_start(out=xT_sb[:, :, 1024:1536],
                              in_=xT.ap()[:, :, 1024:1536])
            nc.scalar.dma_start(out=wo_sb, in_=wo.ap())
            nc.sync.dma_start(out=xT_sb[:, :, 1536:2048],
                              in_=xT.ap()[:, :, 1536:2048])

            nc.gpsimd.memset(zz_sb, 0.0)

            pG_t = pG.tile([128, 512], F32, name="gaug")
            EXM = {}
            EXS = {}
            gav_first = [True]

            # ---------------- filler plumbing ----------------
            fillq = deque()
            done_gens = set()

            def fill(n):
                while n > 0 and fillq:
                    try:
                        next(fillq[0])
                        n -= 1
                    except StopIteration:
                        done_gens.add(fillq.popleft())

            def force_through(g):
                if g in done_gens:
                    return
                while fillq and fillq[0] is not g:
                    gh = fillq.popleft()
                    for _ in gh:
                        pass
                    done_gens.add(gh)
                if fillq and fillq[0] is g:
                    fillq.popleft()
                for _ in g:
                    pass
                done_gens.add(g)

            # ---------------- unit emitters ----------------
            def qk_gen(st, w_sb, b_sb, dst, nm):
                ssl = slice(512 * st, 512 * (st + 1))
                for cbp in (0, 1):
                    acc = [pA.tile([128, 512], F32, tag="pa",
                                   name=f"{nm}{st}_{cbp}_{i}") for i in (0, 1)]
                    for dc in range(8):
                        for i in (0, 1):
                            cb = 2 * cbp + i
                            nc.tensor.matmul(
                                acc[i], w_sb[:, dc, 128 * cb:128 * (cb + 1)],
                                xT_sb[:, dc, ssl],
                                start=(dc == 0), stop=(dc == 7))
                        yield
                    for i in (0, 1):
                        cb = 2 * cbp + i
                        nc.scalar.activation(
                            out=dst[:, cb, ssl], in_=acc[i], func=AF.Identity,
                            bias=b_sb[:, cb:cb + 1], scale=1.0)
                        yield

            def v_gen(st):
                s0 = 512 * st
                for s4p in (0, 1):
                    pv = [pA.tile([128, 512], F32, tag="pa",
                                  name=f"pv{st}_{s4p}_{i}") for i in (0, 1)]
                    for dc in range(8):
                        for i in (0, 1):
                            s4 = 2 * s4p + i
                            nc.tensor.matmul(
                                pv[i],
                                xT_sb[:, dc, s0 + 128 * s4:s0 + 128 * (s4 + 1)],
                                wv_sb[:, dc, :],
                                start=(dc == 0), stop=(dc == 7))
                        yield
                    for i in (0, 1):
                        sc = 4 * st + 2 * s4p + i
                        vview = v_sb[:, sc, :].rearrange("p (h w) -> p h w", h=8)
                        nc.vector.tensor_add(
                            out=vview[:, :, 0:64],
                            in0=pv[i].rearrange("p (h w) -> p h w", h=8),
                            in1=bv_sb.rearrange("p (h w) -> p h w", h=8))
                        nc.gpsimd.memset(vview[:, :, 64:65], 1.0)
                        yield

            def wave(P):
                wA, wB, Wt = pair_widths(P)
                moff = msk_off(P)
                for hp in range(4):
                    bk = {hs: pSC.tile([128, 512], F32, tag="sc",
                                       name=f"sc{P}_{hp}_{hs}")
                          for hs in (0, 64)}
                    for par, (kk, w, co) in enumerate(
                            [(2 * P, wA, 0), (2 * P + 1, wB, wA)]):
                        s0, _ = chunk_sw(kk)
                        for hs in (0, 64):
                            lhs = k_sb[hs:hs + 64, hp, 128 * kk:128 * kk + 128]
                            nc.tensor.matmul(
                                bk[hs][:, co:co + w], lhs,
                                q_sb[hs:hs + 64, hp, s0:s0 + w],
                                start=(par == 0), stop=False,
                                skip_group_check=True)
                            nc.tensor.matmul(
                                bk[hs][:, wA + wB + 4 * par:wA + wB + 4 * par + 4],
                                lhs, q_sb[hs:hs + 64, hp, 0:4],
                                start=False, stop=(par == 1),
                                skip_group_check=True)
                    ex = epool.tile([128, 2 * Wt], BF16, tag="ex", bufs=4,
                                    name=f"ex{P}_{hp}")
                    for h64, hs in enumerate((0, 64)):
                        nc.scalar.activation(
                            out=ex[:, h64 * Wt:(h64 + 1) * Wt],
                            in_=bk[hs][:, 0:Wt], func=AF.Exp, scale=SCALE)
                    exm = epool.tile([128, 2 * Wt], BF16, tag="exm", bufs=20,
                                     name=f"xm{P}_{hp}")
                    nc.vector.tensor_mul(out=exm, in0=ex,
                                         in1=masks_sb[:, moff:moff + 2 * Wt])
                    EXM[(P, hp)] = (exm, Wt, wA)
                    fill(2)

            def strips(b):
                c0 = 160 if b == 0 else 0
                exs = epool.tile([100, 2, 512], BF16, tag="exs", bufs=2,
                                 name=f"exs{b}")
                for sb in (0, 1):
                    bank = pSC.tile([128, 512], F32, tag="sc",
                                    name=f"stb{b}_{sb}")
                    nc.tensor.matmul(bank[0:100, :], zz_sb[0:1, 0:100],
                                     zz_sb[0:1, 0:512],
                                     start=True, stop=False,
                                     skip_group_check=True)
                    for j, (hp, hs) in enumerate(
                            [(2 * sb, 0), (2 * sb, 64),
                             (2 * sb + 1, 0), (2 * sb + 1, 64)]):
                        nc.tensor.matmul(
                            bank[32 * j:32 * j + 4, c0:512],
                            k_sb[hs:hs + 64, hp, 0:4],
                            q_sb[hs:hs + 64, hp, 512 * b + c0:512 * (b + 1)],
                            start=False, stop=(j == 3),
                            skip_group_check=True,
                            tile_position=(hs, 32 * j))
                    nc.scalar.activation(out=exs[:, sb, :],
                                         in_=bank[0:100, 0:512],
                                         func=AF.Exp, scale=SCALE)
                    fill(2)
                EXS[b] = exs

            def gav(kk):
                P, par = divmod(kk, 2)
                for hp in range(4):
                    exm, Wt, wA = EXM[(P, hp)]
                    for h64 in (0, 1):
                        gcol = 8 * hp + 4 * h64
                        h65 = (2 * hp + h64) * 65
                        nc.tensor.matmul(
                            pG_t[0:65, gcol:gcol + 4],
                            v_sb[:, kk, h65:h65 + 65],
                            exm[:, Wt * h64 + Wt - 8 + 4 * par:
                                Wt * h64 + Wt - 8 + 4 * par + 4],
                            start=gav_first[0], stop=(kk == 15),
                            skip_group_check=True)
                        gav_first[0] = False
                    fill(1)

            def a_unit(st, hp, hs):
                B = 512 * st
                c0 = 4 if st == 0 else 0
                h64 = hs // 64
                h65 = (2 * hp + h64) * 65
                aug = pAU.tile([128, 512], F32, tag="aug",
                               name=f"au{st}_{hp}_{hs}")
                sb, j = hp // 2, 2 * (hp % 2) + h64
                sc0 = 160 if st == 0 else 0
                nc.tensor.matmul(
                    aug[0:65, sc0:512],
                    vg_sb[32 * j:32 * j + 4, h65:h65 + 65],
                    EXS[st][32 * j:32 * j + 4, sb, sc0:512],
                    start=True, stop=False, skip_group_check=True,
                    tile_position=(32 * j, 0))
                chunks = av_chunks(st)
                for i, (P, par, lo, w, acol) in enumerate(chunks):
                    exm, Wt, wA = EXM[(P, hp)]
                    base = wA if par else 0
                    kk = 2 * P + par
                    nc.tensor.matmul(
                        aug[0:65, acol:acol + w],
                        v_sb[:, kk, h65:h65 + 65],
                        exm[:, Wt * h64 + base + lo:Wt * h64 + base + lo + w],
                        start=False, stop=(i == len(chunks) - 1),
                        skip_group_check=True)
                sums = rpool.tile([1, 512], F32, tag="sums", bufs=3,
                                  name=f"su{st}_{hp}_{hs}")
                nc.vector.tensor_copy(out=sums, in_=aug[64:65, 0:512])
                rec = rpool.tile([1, 512], F32, tag="rec", bufs=3,
                                 name=f"re{st}_{hp}_{hs}")
                nc.vector.reciprocal_approx_fast(out=rec, in_=sums)
                bc = rpool.tile([64, 512], F32, tag="bc", bufs=3,
                                name=f"bc{st}_{hp}_{hs}")
                nc.gpsimd.partition_broadcast(bc, rec)
                nc.vector.tensor_mul(
                    out=att_sb[hs:hs + 64, hp, B + c0:B + 512],
                    in0=aug[0:64, c0:512], in1=bc[0:64, c0:512])

            def a_phase(st):
                for hp in range(4):
                    for hs in (0, 64):
                        a_unit(st, hp, hs)
                        fill(3)

            def op_pair_gen(st, e2):
                B = 512 * st
                c0 = 4 if st == 0 else 0
                otq = opool.tile([128, 2, 512], BF16, tag="otq", bufs=3,
                                 name=f"otq{st}_{e2}")
                for i in (0, 1):
                    et = 2 * e2 + i
                    po = pA.tile([128, 512], F32, tag="pa",
                                 name=f"po{st}_{et}")
                    for cb in range(4):
                        nc.tensor.matmul(
                            po[:, c0:512], wo_sb[:, cb, 128 * et:128 * et + 128],
                            att_sb[:, cb, B + c0:B + 512],
                            start=(cb == 0), stop=(cb == 3))
                        if cb % 2:
                            yield
                    if et % 2:
                        nc.scalar.activation(out=otq[:, i, c0:512],
                                             in_=po[:, c0:512], func=AF.Identity)
                    else:
                        nc.vector.tensor_copy(out=otq[:, i, c0:512],
                                              in_=po[:, c0:512])
                    yield
                eng = nc.sync if (e2 % 2 == 0) else nc.scalar
                eng.dma_start(out=out.ap()[:, 2 * e2:2 * e2 + 2, B + c0:B + 512],
                              in_=otq[:, :, c0:512])
                yield

            def g_norm():
                for hp in range(4):
                    for h64, hs in enumerate((0, 64)):
                        gcol = 8 * hp + 4 * h64
                        sg = rpool.tile([1, 4], F32, tag="sumg", bufs=8,
                                        name=f"sg{hp}_{h64}")
                        nc.vector.tensor_copy(out=sg,
                                              in_=pG_t[64:65, gcol:gcol + 4])
                        rg = rpool.tile([1, 4], F32, tag="recg", bufs=8,
                                        name=f"rg{hp}_{h64}")
                        nc.vector.reciprocal_approx_fast(out=rg, in_=sg)
                        bg = rpool.tile([64, 4], F32, tag="bcg", bufs=8,
                                        name=f"bg{hp}_{h64}")
                        nc.gpsimd.partition_broadcast(bg, rg)
                        nc.vector.tensor_mul(
                            out=att_sb[hs:hs + 64, hp, 0:4],
                            in0=pG_t[0:64, gcol:gcol + 4], in1=bg)
                        fill(1)

            def og():
                po = pA.tile([128, 512], F32, tag="pa", name="pog")
                for et in range(8):
                    for cb in range(4):
                        nc.tensor.matmul(
                            po[:, 4 * et:4 * et + 4],
                            wo_sb[:, cb, 128 * et:128 * et + 128],
                            att_sb[:, cb, 0:4],
                            start=(cb == 0), stop=(cb == 3),
                            skip_group_check=True)
                    fill(1)
                otg = opool.tile([128, 32], BF16, tag="otg", bufs=1,
                                 name="otg")
                nc.vector.tensor_copy(out=otg, in_=po[:, 0:32])
                nc.sync.dma_start(
                    out=out.ap()[:, :, 0:4],
                    in_=otg.rearrange("p (e g) -> p e g", e=8))

            # ---------------- schedule ----------------
            K = [qk_gen(st, wk_sb, bk_sb, k_sb, "k") for st in range(4)]
            Q = [qk_gen(st, wq_sb, bq_sb, q_sb, "q") for st in range(4)]
            V = [v_gen(st) for st in range(4)]
            fillq.extend([K[0], Q[0], V[0], K[1], Q[1], V[1],
                          K[2], Q[2], V[2], K[3], Q[3], V[3]])

            force_through(K[0])
            force_through(Q[0])
            wave(0)
            strips(0)
            force_through(V[0])
            for j in range(4):
                eng = nc.sync if j % 2 == 0 else nc.scalar
                eng.dma_start(out=vg_sb[32 * j:32 * j + 4, :],
                              in_=v_sb[0:4, 0, :])
            gav(0)
            gav(1)
            force_through(K[1])
            force_through(Q[1])
            wave(1)
            gav(2)
            gav(3)
            wave(2)
            strips(1)
            force_through(V[1])
            gav(4)
            gav(5)
            a_phase(0)
            force_through(K[2])
            force_through(Q[2])
            wave(3)
            gav(6)
            gav(7)
            wave(4)
            strips(2)
            force_through(V[2])
            gav(8)
            gav(9)
            fillq.extend([op_pair_gen(0, e2) for e2 in range(4)])
            a_phase(1)
            force_through(K[3])
            force_through(Q[3])
            wave(5)
            gav(10)
            gav(11)
            wave(6)
            strips(3)
            force_through(V[3])
            gav(12)
            gav(13)
            wave(7)
            gav(14)
            gav(15)
            fillq.extend([op_pair_gen(1, e2) for e2 in range(4)])
            a_phase(2)
            g_norm()
            fillq.extend([op_pair_gen(2, e2) for e2 in range(4)])
            a_phase(3)
            og()
            fillq.extend([op_pair_gen(3, e2) for e2 in range(4)])
            while fillq:
                fill(100)

            if DBG == "att":
                nc.sync.dma_start(out=dbg.ap(), in_=att_sb)
            elif DBG == "q":
                nc.sync.dma_start(out=dbg.ap(), in_=q_sb)
            elif DBG == "k":
                nc.sync.dma_start(out=dbg.ap(), in_=k_sb)
            elif DBG == "v":
                nc.sync.dma_start(out=dbg.ap()[:, :, 0:520], in_=v_sb[:, 0:4, :])
                nc.sync.dma_start(out=dbg.ap()[:, :, 520:1040], in_=v_sb[:, 4:8, :])
                nc.sync.dma_start(out=dbg.ap()[:, :, 1040:1560], in_=v_sb[:, 8:12, :])
                nc.sync.dma_start(out=dbg.ap()[:, :, 1560:2048], in_=v_sb[:, 12:16, 0:488])

    nc.compile()
    return nc


def _host_masks():
    p = np.arange(128)[:, None]
    ji = np.arange(192)[None, :]
    mint = (np.abs(p + 32 - ji) <= 32).astype(np.float32)
    j0 = np.arange(160)[None, :]
    m0 = ((j0 >= 4) & ((np.abs(p - j0) <= 32) | (p < 4))).astype(np.float32)
    ones8 = np.ones((128, 8), np.float32)
    P0 = np.concatenate([m0, mint, ones8], 1)
    PI = np.concatenate([mint, mint, ones8], 1)
    P7 = np.concatenate([mint, mint[:, 0:160], ones8], 1)
    full = np.concatenate([P0, P0, PI, PI, P7, P7], 1)
    assert full.shape[1] == MSK_W
    return full.astype(ml_dtypes.bfloat16)


_NC = None
_LAST_IN_MAPS = None
_LAST_RES = None


def kernel(x, Wq, bq, Wk, bk, Wv, bv, Wo, bo):
    global _NC
    if _NC is None:
        _NC = build_nc()
    nc = _NC
    x = np.asarray(x, np.float32)
    B = x.shape[0]
    bf = ml_dtypes.bfloat16

    def chunked_T(a):  # [R, C] -> [128, C//128, R]; [p, c, r] = a[r, 128c+p]
        at = np.ascontiguousarray(a.T)
        return at.reshape(at.shape[0] // 128, 128, at.shape[1]).transpose(1, 0, 2)

    masks_h = _host_masks()
    in_maps = []
    for core in range(NCORES):
        b, g = core // 2, core % 2
        gs = slice(512 * g, 512 * (g + 1))
        in_maps.append({
            "xT": np.ascontiguousarray(chunked_T(x[b])).astype(bf),
            "wq": np.ascontiguousarray(chunked_T(np.asarray(Wq)[gs, :])).astype(bf),
            "wk": np.ascontiguousarray(chunked_T(np.asarray(Wk)[gs, :])).astype(bf),
            "wv": np.ascontiguousarray(chunked_T(np.asarray(Wv)[gs, :])).astype(bf),
            "wo": np.ascontiguousarray(chunked_T(np.asarray(Wo)[:, gs])).astype(bf),
            "bq_c": np.asarray(bq)[gs].reshape(4, 128).T.copy().astype(np.float32),
            "bk_c": np.asarray(bk)[gs].reshape(4, 128).T.copy().astype(np.float32),
            "bv_b": np.broadcast_to(
                np.asarray(bv)[gs], (128, 512)).copy().astype(np.float32),
            "masks": masks_h,
        })

    global _LAST_IN_MAPS, _LAST_RES
    _LAST_IN_MAPS = in_maps
    res = run_bass_kernel_spmd(nc, in_maps, list(range(NCORES)))
    _LAST_RES = res
    out = np.empty((B, S, D), np.float32)
    for b in range(B):
        acc = res.results[2 * b]["out"].astype(np.float32) + \
            res.results[2 * b + 1]["out"].astype(np.float32)
        full_T = acc.transpose(1, 0, 2).reshape(D, S)
        out[b] = full_T.T + np.asarray(bo)[None, :]
    return out
